# revision 16
# baseline (speedup 1.0000x reference)
# Trainium2 Bass kernel for nn_Model_26190710571339 (topk_masking).
#
# Model: scores = einsum('bnf,f->bn', feats, w_conv); per-bag sort -> bottom-5
# and top-5 score values -> tiny MLP (10->200->100->1, sigmoid) -> logits, probs.
#
# Sharding: data-parallel over the bag axis; 2 bags per NeuronCore x 8 cores.
# Weights replicated.
#
# v6: hybrid DVE + PE scoring. feats are cast to bf16 on the host (halves the
# HBM stream to 134 MB/core; measured rel err 7e-4 vs the 2e-2 gate). The DVE
# fused multiply+reduce runs at 1x (2284 ns per 2048-wide tile), so DVE alone
# binds at ~585 us. To break that, each bag's rows are split on the host:
#   - rows [0, R_DB):   normal layout, DVE scalar_tensor_tensor (as v5)
#   - rows [R_DB, 16384): HOST-TRANSPOSED layout [16 fchunk, 128 f, rows];
#     the PE computes scores: stationary lhsT = w-chunk replicated into
#     [128 f, 128] (every column identical), moving rhs = [128 f, 512 rows],
#     PSUM accumulates over the 16 f-chunks. Every PSUM partition then holds
#     the same 512 scores, so one ACT copy of PSUM partition 0 to an SBUF row
#     plus one partition-unfold DMA scatters each group bijectively into the
#     same [128, cols] scores tile the DVE writes -> the topk/MLP tail is
#     unchanged.
# DVE ~290 us and PE ~270 us run in parallel (measured 398-429 us total).
#   - per-bag top-8/bottom-8 via single-instruction nc.vector.max (top-5 of a
#     bag is always inside the union of per-partition top-8), bottom side via
#     max over negated scores; bag 0 overlaps the bag-1 stream
#   - sort-order permutation + bottom-side sign flip folded into W1 on host
#   - MLP in transposed form; logits+probs packed into one [1, 4] output DMA
# (tensor_tensor_reduce crashes the device in this lowering; gpsimd cannot
# run stt [Pool engine opcode check]; bf16 stt runs at DVE 1x mode.)

import numpy as np

B = 16
NTILES = 16384
FSZ = 2048
R = 5
NCORES = 8
BAGS_PER_CORE = B // NCORES  # 2


def _split(ntiles):
    """Per-bag row split between the DVE and PE pipelines."""
    r_pb = ntiles // 2           # PE rows per bag; r_pb/128 must divide 512
    srs = min(2048, r_pb)        # rows per PE "super" (PSUM residency unit)
    return ntiles - r_pb, r_pb, srs


def _build_nc(nbags, ntiles, fsz, ncores=NCORES):
    import concourse.mybir as mybir
    import concourse.tile as tile
    from concourse import bacc
    from contextlib import ExitStack

    f32 = mybir.dt.float32
    bf16 = mybir.dt.bfloat16
    Alu = mybir.AluOpType
    Act = mybir.ActivationFunctionType

    r_db, r_pb, srs = _split(ntiles)
    nfc = fsz // 128              # f-chunks (16)
    nchunk_d = r_db // 256        # DVE 1MB chunks per bag
    supers = r_pb // srs          # PE supers per bag
    gps = srs // 512              # 512-row PSUM groups per super
    groups = r_pb // 512          # groups per bag
    X = r_pb // 128               # PE score cols per partition; must divide 512
    assert 512 % X == 0
    pieces = 512 // X             # copy pieces per group
    cols_per_bag = ntiles // 128
    cols_d = r_db // 128          # DVE score cols per bag
    nblk = nbags * cols_per_bag

    nc = bacc.Bacc("TRN2", target_bir_lowering=False, debug=False, num_devices=ncores)
    feats_d = nc.declare_dram_parameter(
        "feats_d", [nbags * nchunk_d, 128, 4096], bf16, isOutput=False)
    feats_p = nc.declare_dram_parameter(
        "feats_p", [nbags * supers * nfc, 128, srs], bf16, isOutput=False)
    wb = nc.declare_dram_parameter("wb", [128, fsz], bf16, isOutput=False)
    wrep = nc.declare_dram_parameter("wrep", [128, nfc * 128], bf16, isOutput=False)
    w1t = nc.declare_dram_parameter("w1t", [16, 200], f32, isOutput=False)
    w2ta = nc.declare_dram_parameter("w2ta", [128, 100], f32, isOutput=False)
    w2tb = nc.declare_dram_parameter("w2tb", [72, 100], f32, isOutput=False)
    w3t = nc.declare_dram_parameter("w3t", [100, 1], f32, isOutput=False)
    b1a = nc.declare_dram_parameter("b1a", [128, 1], f32, isOutput=False)
    b1b = nc.declare_dram_parameter("b1b", [72, 1], f32, isOutput=False)
    b2c = nc.declare_dram_parameter("b2c", [100, 1], f32, isOutput=False)
    b3c = nc.declare_dram_parameter("b3c", [1, 1], f32, isOutput=False)
    idn = nc.declare_dram_parameter("idn", [nbags, nbags], f32, isOutput=False)
    outlp = nc.declare_dram_parameter("outlp", [1, 2 * nbags], f32, isOutput=True)

    dma_rr = [0]

    def next_ring():
        dma_rr[0] += 1
        return nc.sync if dma_rr[0] % 2 == 0 else nc.scalar

    with ExitStack() as ctx:
        tc = ctx.enter_context(tile.TileContext(nc))
        consts = ctx.enter_context(tc.tile_pool(name="consts", bufs=1))

        wb_sb = consts.tile([128, fsz], bf16)
        nc.sync.dma_start(wb_sb[:], wb[:])
        wrep_sb = consts.tile([128, nfc * 128], bf16)
        nc.scalar.dma_start(wrep_sb[:], wrep[:])
        scores = consts.tile([128, nblk], f32)

        # per-bag selection state
        tpool = ctx.enter_context(tc.tile_pool(name="tpool", bufs=1))
        neg = tpool.tile([128, cols_per_bag], f32)
        gsrc = [tpool.tile([128, 16], f32, name=f"gsrc{b}") for b in range(nbags)]
        cand_top = tpool.tile([nbags, 128 * 8], f32)
        cand_bot = tpool.tile([nbags, 128 * 8], f32)
        mm = tpool.tile([nbags, 16], f32)

        def emit_bag_topk(b):
            sc_b = scores[:, b * cols_per_bag : (b + 1) * cols_per_bag]
            # top-8 per partition; bottom-8 via max over negated scores
            nc.vector.max(gsrc[b][:, 0:8], sc_b)
            nc.vector.tensor_scalar_mul(neg[:], sc_b, -1.0)
            nc.vector.max(gsrc[b][:, 8:16], neg[:])
            # gather the 128x8 candidates of each side into one partition row
            nc.scalar.dma_start(cand_top[b : b + 1, :], gsrc[b][:, 0:8])
            nc.sync.dma_start(cand_bot[b : b + 1, :], gsrc[b][:, 8:16])

        fpool = ctx.enter_context(tc.tile_pool(name="fpool", bufs=4))
        opool = ctx.enter_context(tc.tile_pool(name="opool", bufs=4))
        spool = ctx.enter_context(tc.tile_pool(name="spool", bufs=20))
        prpool = ctx.enter_context(tc.tile_pool(name="prpool", bufs=4))
        pe_psum = ctx.enter_context(
            tc.tile_pool(name="pe_psum", bufs=3, space="PSUM"))

        def emit_dve_chunk(b, k):
            ft = fpool.tile([128, 4096], bf16, name="ft")
            next_ring().dma_start(ft[:], feats_d[b * nchunk_d + k])
            for h in range(2):
                col = b * cols_per_bag + 2 * k + h
                ot = opool.tile([128, fsz], bf16, name="ot")
                nc.vector.scalar_tensor_tensor(
                    out=ot[:],
                    in0=ft[:, h * fsz : (h + 1) * fsz],
                    scalar=1.0,
                    in1=wb_sb[:],
                    op0=Alu.mult,
                    op1=Alu.mult,
                    accum_out=scores[:, col : col + 1],
                )

        def emit_pe_super(b, s):
            slabs = []
            for c in range(nfc):
                sl = spool.tile([128, srs], bf16, name="sl")
                next_ring().dma_start(sl[:], feats_p[(b * supers + s) * nfc + c])
                slabs.append(sl)
            # group-outer: only group 0's c-walk chases the slab DMAs; later
            # groups re-read resident slabs with no waits, keeping PE dense.
            pr = prpool.tile([1, gps * 512], f32, name="pr")
            for g in range(gps):
                ps = pe_psum.tile([128, 512], f32, name="ps")
                for c in range(nfc):
                    nc.tensor.matmul(
                        ps[:], lhsT=wrep_sb[:, c * 128 : (c + 1) * 128],
                        rhs=slabs[c][:, g * 512 : (g + 1) * 512],
                        start=(c == 0), stop=(c == nfc - 1),
                    )
                # every PSUM partition holds the group's 512 scores; stage
                # partition 0 into the super's SBUF row
                nc.scalar.activation(
                    pr[:, g * 512 : (g + 1) * 512], ps[0:1, :], Act.Copy)
            # one partition-unfold DMA scatters the super's gps*512 scores
            # over partitions [pieces*gps*s, pieces*gps*(s+1)) x X cols of
            # this bag's PE score region
            qps = pieces * gps
            next_ring().dma_start(
                scores[qps * s : qps * (s + 1),
                       b * cols_per_bag + cols_d :
                       b * cols_per_bag + cols_d + X],
                pr[:],
            )

        # ---- main loop: interleave DVE chunks and PE supers per bag
        cps = nchunk_d // supers  # DVE chunks emitted per PE super
        for b in range(nbags):
            for s in range(supers):
                for k in range(s * cps, (s + 1) * cps):
                    emit_dve_chunk(b, k)
                emit_pe_super(b, s)
            emit_bag_topk(b)

        w1t_sb = consts.tile([16, 200], f32)
        nc.sync.dma_start(w1t_sb[:], w1t[:])
        w2ta_sb = consts.tile([128, 100], f32)
        nc.sync.dma_start(w2ta_sb[:], w2ta[:])
        w2tb_sb = consts.tile([72, 100], f32)
        nc.sync.dma_start(w2tb_sb[:], w2tb[:])
        w3t_sb = consts.tile([100, 1], f32)
        nc.sync.dma_start(w3t_sb[:], w3t[:])
        b1a_sb = consts.tile([128, 1], f32)
        nc.sync.dma_start(b1a_sb[:], b1a[:])
        b1b_sb = consts.tile([72, 1], f32)
        nc.sync.dma_start(b1b_sb[:], b1b[:])
        b2c_sb = consts.tile([100, 1], f32)
        nc.sync.dma_start(b2c_sb[:], b2c[:])
        b3c_sb = consts.tile([1, 1], f32)
        nc.sync.dma_start(b3c_sb[:], b3c[:])
        idn_sb = consts.tile([nbags, nbags], f32)
        nc.sync.dma_start(idn_sb[:], idn[:])

        # ---- global top/bottom-8 per bag (rows = bags); top-5 subset exact
        nc.vector.max(mm[:, 0:8], cand_top[:])
        nc.vector.max(mm[:, 8:16], cand_bot[:])

        # ---- MLP (transposed): hT = sigmoid(W @ xT + b), biases per-partition.
        # mm columns = [top8 desc | (-bottom)8 desc]; W1 permuted/sign-flipped
        # on the host to match, so this equals the reference MLP.
        psum = ctx.enter_context(tc.tile_pool(name="psum", bufs=1, space="PSUM"))

        mmT_ps = psum.tile([16, nbags], f32, name="mmT_ps")[:]
        h1pa = psum.tile([128, nbags], f32, name="h1pa")[:]
        h1pb = psum.tile([72, nbags], f32, name="h1pb")[:]
        h2p = psum.tile([100, nbags], f32, name="h2p")[:]
        lp = psum.tile([1, nbags], f32, name="lp")[:]
        nc.tensor.transpose(mmT_ps, mm[:], idn_sb[:])
        mmT = tpool.tile([16, nbags], f32)
        nc.vector.tensor_copy(mmT[:], mmT_ps)

        nc.tensor.matmul(h1pa, lhsT=w1t_sb[:, 0:128], rhs=mmT[:], start=True, stop=True)
        nc.tensor.matmul(h1pb, lhsT=w1t_sb[:, 128:200], rhs=mmT[:], start=True, stop=True)
        h1a = tpool.tile([128, nbags], f32)
        h1b = tpool.tile([72, nbags], f32)
        nc.scalar.activation(h1a[:], h1pa, Act.Sigmoid, bias=b1a_sb[:], scale=1.0)
        nc.scalar.activation(h1b[:], h1pb, Act.Sigmoid, bias=b1b_sb[:], scale=1.0)

        nc.tensor.matmul(h2p, lhsT=w2ta_sb[:], rhs=h1a[:], start=True, stop=False)
        nc.tensor.matmul(h2p, lhsT=w2tb_sb[:], rhs=h1b[:], start=False, stop=True)
        h2 = tpool.tile([100, nbags], f32)
        nc.scalar.activation(h2[:], h2p, Act.Sigmoid, bias=b2c_sb[:], scale=1.0)

        nc.tensor.matmul(lp, lhsT=w3t_sb[:], rhs=h2[:], start=True, stop=True)
        outt = tpool.tile([1, 2 * nbags], f32)
        nc.vector.tensor_scalar_add(outt[:, 0:nbags], lp, b3c_sb[:])
        nc.scalar.activation(outt[:, nbags : 2 * nbags], outt[:, 0:nbags], Act.Sigmoid)

        nc.sync.dma_start(outlp[:], outt[:])

    nc.finalize()
    return nc


def _make_in_maps(inputs, nbags, ntiles, fsz, ncores):
    import ml_dtypes

    bf16 = ml_dtypes.bfloat16
    feats = np.asarray(inputs["feats"], dtype=np.float32)
    w_conv = np.asarray(inputs["w_conv"], dtype=np.float32)
    W1 = np.asarray(inputs["W1"], dtype=np.float32)
    b1 = np.asarray(inputs["b1"], dtype=np.float32)
    W2 = np.asarray(inputs["W2"], dtype=np.float32)
    b2 = np.asarray(inputs["b2"], dtype=np.float32)
    W3 = np.asarray(inputs["W3"], dtype=np.float32)
    b3 = np.asarray(inputs["b3"], dtype=np.float32)

    r_db, r_pb, srs = _split(ntiles)
    nfc = fsz // 128
    nchunk_d = r_db // 256
    supers = r_pb // srs

    # kernel x layout: x[k] = k-th largest score (k<8), x[8+j] = j-th largest
    # of negated scores = -(j-th smallest) (j<8).
    # reference minmax: [b0..b4 asc, t0..t4 asc] -> W1p columns:
    W1p = np.zeros((200, 16), dtype=np.float32)
    for k in range(5):
        W1p[:, k] = W1[:, 9 - k]          # t_(k) -> minmax[9-k]
    for j in range(5):
        W1p[:, 8 + j] = -W1[:, j]         # -b_j -> minmax[j]

    w16 = w_conv.astype(bf16)
    wrep = np.zeros((128, nfc * 128), dtype=bf16)
    for c in range(nfc):
        wrep[:, c * 128 : (c + 1) * 128] = w16[c * 128 : (c + 1) * 128][:, None]

    base = {
        "wb": np.ascontiguousarray(np.broadcast_to(w16, (128, fsz))),
        "wrep": wrep,
        "w1t": np.ascontiguousarray(W1p.T),
        "w2ta": np.ascontiguousarray(W2.T[:128]),
        "w2tb": np.ascontiguousarray(W2.T[128:]),
        "w3t": np.ascontiguousarray(W3.T),
        "b1a": np.ascontiguousarray(b1[:128].reshape(128, 1)),
        "b1b": np.ascontiguousarray(b1[128:].reshape(72, 1)),
        "b2c": np.ascontiguousarray(b2.reshape(100, 1)),
        "b3c": np.ascontiguousarray(b3.reshape(1, 1)),
        "idn": np.eye(nbags, dtype=np.float32),
    }
    feats16 = feats.astype(bf16)
    in_maps = []
    for cidx in range(ncores):
        shard = feats16[cidx * nbags : (cidx + 1) * nbags]  # [nbags, ntiles, fsz]
        fd = shard[:, :r_db, :].reshape(nbags * nchunk_d, 128, 4096)
        # PE part: [b, super, fchunk, 128 f, srs rows]
        fp = (shard[:, r_db:, :]
              .reshape(nbags, supers, srs, nfc, 128)
              .transpose(0, 1, 3, 4, 2)
              .reshape(nbags * supers * nfc, 128, srs))
        in_maps.append({
            **base,
            "feats_d": np.ascontiguousarray(fd),
            "feats_p": np.ascontiguousarray(fp),
        })
    return in_maps


def _run(inputs, trace=False, **spmd_kwargs):
    from concourse.bass_utils import run_bass_kernel_spmd

    nc = _build_nc(BAGS_PER_CORE, NTILES, FSZ)
    in_maps = _make_in_maps(inputs, BAGS_PER_CORE, NTILES, FSZ, NCORES)
    res = run_bass_kernel_spmd(
        nc, in_maps, list(range(NCORES)), trace=trace, **spmd_kwargs
    )
    logits = np.concatenate(
        [res.results[c]["outlp"].reshape(2, BAGS_PER_CORE)[0].reshape(-1, 1)
         for c in range(NCORES)],
        axis=0,
    )
    probs = np.concatenate(
        [res.results[c]["outlp"].reshape(2, BAGS_PER_CORE)[1].reshape(-1, 1)
         for c in range(NCORES)],
        axis=0,
    )
    return (logits, probs), res


def kernel(**inputs):
    out, _ = _run(inputs, trace=False)
    return out


# revision 19
# speedup vs baseline: 1.0278x; 1.0278x over previous
# Trainium2 Bass kernel for nn_Model_26190710571339 (topk_masking).
#
# Model: scores = einsum('bnf,f->bn', feats, w_conv); per-bag sort -> bottom-5
# and top-5 score values -> tiny MLP (10->200->100->1, sigmoid) -> logits, probs.
#
# Sharding: data-parallel over the bag axis; 2 bags per NeuronCore x 8 cores.
# Weights replicated.
#
# v6: hybrid DVE + PE scoring. feats are cast to bf16 on the host (halves the
# HBM stream to 134 MB/core; measured rel err 7e-4 vs the 2e-2 gate). The DVE
# fused multiply+reduce runs at 1x (2284 ns per 2048-wide tile), so DVE alone
# binds at ~585 us. To break that, each bag's rows are split on the host:
#   - rows [0, R_DB):   normal layout, DVE scalar_tensor_tensor (as v5)
#   - rows [R_DB, 16384): HOST-TRANSPOSED layout [16 fchunk, 128 f, rows];
#     the PE computes scores: stationary lhsT = w-chunk replicated into
#     [128 f, 128] (every column identical), moving rhs = [128 f, 512 rows],
#     PSUM accumulates over the 16 f-chunks. Every PSUM partition then holds
#     the same 512 scores, so one ACT copy of PSUM partition 0 to an SBUF row
#     plus one partition-unfold DMA scatters each group bijectively into the
#     same [128, cols] scores tile the DVE writes -> the topk/MLP tail is
#     unchanged.
# DVE ~290 us and PE ~270 us run in parallel (measured 398-429 us total).
#   - per-bag top-8/bottom-8 via single-instruction nc.vector.max (top-5 of a
#     bag is always inside the union of per-partition top-8), bottom side via
#     max over negated scores; bag 0 overlaps the bag-1 stream
#   - sort-order permutation + bottom-side sign flip folded into W1 on host
#   - MLP in transposed form; logits+probs packed into one [1, 4] output DMA
# (tensor_tensor_reduce crashes the device in this lowering; gpsimd cannot
# run stt [Pool engine opcode check]; bf16 stt runs at DVE 1x mode.)

import numpy as np

B = 16
NTILES = 16384
FSZ = 2048
R = 5
NCORES = 8
BAGS_PER_CORE = B // NCORES  # 2


def _split(ntiles):
    """Per-bag row split between the DVE and PE pipelines."""
    if ntiles >= 16384:
        return 6144, 10240, 2560
    r_pb = ntiles // 2
    return ntiles - r_pb, r_pb, 512


def _build_nc(nbags, ntiles, fsz, ncores=NCORES):
    import concourse.mybir as mybir
    import concourse.tile as tile
    from concourse import bacc
    from contextlib import ExitStack

    f32 = mybir.dt.float32
    bf16 = mybir.dt.bfloat16
    Alu = mybir.AluOpType
    Act = mybir.ActivationFunctionType

    r_db, r_pb, srs = _split(ntiles)
    nfc = fsz // 128              # f-chunks (16)
    nchunk_d = r_db // 256        # DVE 1MB chunks per bag
    supers = r_pb // srs          # PE supers per bag
    gps = srs // 512              # 512-row PSUM groups per super
    X = r_pb // 128               # PE score cols per partition
    assert srs % X == 0
    qps = srs // X                # score partitions covered per super
    cols_per_bag = ntiles // 128
    cols_d = r_db // 128          # DVE score cols per bag
    nblk = nbags * cols_per_bag

    nc = bacc.Bacc("TRN2", target_bir_lowering=False, debug=False, num_devices=ncores)
    feats_d = nc.declare_dram_parameter(
        "feats_d", [nbags * nchunk_d, 128, 4096], bf16, isOutput=False)
    feats_p = nc.declare_dram_parameter(
        "feats_p", [nbags * supers * nfc, 128, srs], bf16, isOutput=False)
    wb = nc.declare_dram_parameter("wb", [128, fsz], bf16, isOutput=False)
    wrep = nc.declare_dram_parameter("wrep", [128, nfc * 128], bf16, isOutput=False)
    w1t = nc.declare_dram_parameter("w1t", [16, 200], f32, isOutput=False)
    w2ta = nc.declare_dram_parameter("w2ta", [128, 100], f32, isOutput=False)
    w2tb = nc.declare_dram_parameter("w2tb", [72, 100], f32, isOutput=False)
    w3t = nc.declare_dram_parameter("w3t", [100, 1], f32, isOutput=False)
    b1a = nc.declare_dram_parameter("b1a", [128, 1], f32, isOutput=False)
    b1b = nc.declare_dram_parameter("b1b", [72, 1], f32, isOutput=False)
    b2c = nc.declare_dram_parameter("b2c", [100, 1], f32, isOutput=False)
    b3c = nc.declare_dram_parameter("b3c", [1, 1], f32, isOutput=False)
    idn = nc.declare_dram_parameter("idn", [nbags, nbags], f32, isOutput=False)
    outlp = nc.declare_dram_parameter("outlp", [1, 2 * nbags], f32, isOutput=True)

    dma_rr = [0]

    def next_ring():
        dma_rr[0] += 1
        return nc.sync if dma_rr[0] % 2 == 0 else nc.scalar

    with ExitStack() as ctx:
        tc = ctx.enter_context(tile.TileContext(nc))
        consts = ctx.enter_context(tc.tile_pool(name="consts", bufs=1))

        wb_sb = consts.tile([128, fsz], bf16)
        nc.sync.dma_start(wb_sb[:], wb[:])
        wrep_sb = consts.tile([128, nfc * 128], bf16)
        nc.scalar.dma_start(wrep_sb[:], wrep[:])
        scores = consts.tile([128, nblk], f32)

        # per-bag selection state
        tpool = ctx.enter_context(tc.tile_pool(name="tpool", bufs=1))
        neg = tpool.tile([128, cols_per_bag], f32)
        gsrc = [tpool.tile([128, 16], f32, name=f"gsrc{b}") for b in range(nbags)]
        cand_top = tpool.tile([nbags, 128 * 8], f32)
        cand_bot = tpool.tile([nbags, 128 * 8], f32)
        mm = tpool.tile([nbags, 16], f32)

        def emit_bag_topk(b):
            sc_b = scores[:, b * cols_per_bag : (b + 1) * cols_per_bag]
            # top-8 per partition; bottom-8 via max over negated scores
            nc.vector.max(gsrc[b][:, 0:8], sc_b)
            nc.vector.tensor_scalar_mul(neg[:], sc_b, -1.0)
            nc.vector.max(gsrc[b][:, 8:16], neg[:])
            # gather the 128x8 candidates of each side into one partition row
            nc.scalar.dma_start(cand_top[b : b + 1, :], gsrc[b][:, 0:8])
            nc.sync.dma_start(cand_bot[b : b + 1, :], gsrc[b][:, 8:16])

        fpool = ctx.enter_context(tc.tile_pool(name="fpool", bufs=4))
        opool = ctx.enter_context(tc.tile_pool(name="opool", bufs=4))
        spool = ctx.enter_context(tc.tile_pool(name="spool", bufs=18))
        prpool = ctx.enter_context(tc.tile_pool(name="prpool", bufs=2))
        pe_psum = ctx.enter_context(
            tc.tile_pool(name="pe_psum", bufs=3, space="PSUM"))

        def emit_dve_chunk(b, k):
            ft = fpool.tile([128, 4096], bf16, name="ft")
            next_ring().dma_start(ft[:], feats_d[b * nchunk_d + k])
            for h in range(2):
                col = b * cols_per_bag + 2 * k + h
                ot = opool.tile([128, fsz], bf16, name="ot")
                nc.vector.scalar_tensor_tensor(
                    out=ot[:],
                    in0=ft[:, h * fsz : (h + 1) * fsz],
                    scalar=1.0,
                    in1=wb_sb[:],
                    op0=Alu.mult,
                    op1=Alu.mult,
                    accum_out=scores[:, col : col + 1],
                )

        def emit_pe_super(b, s):
            slabs = []
            for c in range(nfc):
                sl = spool.tile([128, srs], bf16, name="sl")
                next_ring().dma_start(sl[:], feats_p[(b * supers + s) * nfc + c])
                slabs.append(sl)
            # group-outer: only group 0's c-walk chases the slab DMAs; later
            # groups re-read resident slabs with no waits. Within a group the
            # 16 accumulating matmuls alternate TWO psum banks (even/odd
            # f-chunks) to break the per-matmul PSUM turnaround stall; the
            # two partial sums are added by the scatter DMA (CCE accum).
            pra = prpool.tile([1, gps * 512], f32, name="pra")
            prb = prpool.tile([1, gps * 512], f32, name="prb")
            for g in range(gps):
                psa = pe_psum.tile([128, 512], f32, name="ps")
                psb = pe_psum.tile([128, 512], f32, name="ps")
                for c in range(nfc):
                    ps = psa if c % 2 == 0 else psb
                    nc.tensor.matmul(
                        ps[:], lhsT=wrep_sb[:, c * 128 : (c + 1) * 128],
                        rhs=slabs[c][:, g * 512 : (g + 1) * 512],
                        start=(c < 2), stop=(c >= nfc - 2),
                    )
                # every PSUM partition holds the group's 512 partial scores;
                # stage partition 0 of each bank into the super's SBUF rows
                nc.scalar.activation(
                    pra[:, g * 512 : (g + 1) * 512], psa[0:1, :], Act.Copy)
                nc.scalar.activation(
                    prb[:, g * 512 : (g + 1) * 512], psb[0:1, :], Act.Copy)
            # two partition-unfold DMAs scatter the super's gps*512 scores
            # over partitions [qps*s, qps*(s+1)) x X cols of this bag's PE
            # score region; the second adds the odd-chunk partial in the
            # DMA datapath.
            dst = scores[qps * s : qps * (s + 1),
                         b * cols_per_bag + cols_d :
                         b * cols_per_bag + cols_d + X]
            nc.gpsimd.dma_start(dst, pra[:])
            nc.gpsimd.dma_start(dst, prb[:], accum_op=Alu.add)

        # ---- main loop: interleave DVE chunks and PE supers per bag
        cps = nchunk_d // supers  # DVE chunks emitted per PE super
        for b in range(nbags):
            for s in range(supers):
                for k in range(s * cps, (s + 1) * cps):
                    emit_dve_chunk(b, k)
                emit_pe_super(b, s)
            emit_bag_topk(b)

        w1t_sb = consts.tile([16, 200], f32)
        nc.sync.dma_start(w1t_sb[:], w1t[:])
        w2ta_sb = consts.tile([128, 100], f32)
        nc.sync.dma_start(w2ta_sb[:], w2ta[:])
        w2tb_sb = consts.tile([72, 100], f32)
        nc.sync.dma_start(w2tb_sb[:], w2tb[:])
        w3t_sb = consts.tile([100, 1], f32)
        nc.sync.dma_start(w3t_sb[:], w3t[:])
        b1a_sb = consts.tile([128, 1], f32)
        nc.sync.dma_start(b1a_sb[:], b1a[:])
        b1b_sb = consts.tile([72, 1], f32)
        nc.sync.dma_start(b1b_sb[:], b1b[:])
        b2c_sb = consts.tile([100, 1], f32)
        nc.sync.dma_start(b2c_sb[:], b2c[:])
        b3c_sb = consts.tile([1, 1], f32)
        nc.sync.dma_start(b3c_sb[:], b3c[:])
        idn_sb = consts.tile([nbags, nbags], f32)
        nc.sync.dma_start(idn_sb[:], idn[:])

        # ---- global top/bottom-8 per bag (rows = bags); top-5 subset exact
        nc.vector.max(mm[:, 0:8], cand_top[:])
        nc.vector.max(mm[:, 8:16], cand_bot[:])

        # ---- MLP (transposed): hT = sigmoid(W @ xT + b), biases per-partition.
        # mm columns = [top8 desc | (-bottom)8 desc]; W1 permuted/sign-flipped
        # on the host to match, so this equals the reference MLP.
        psum = ctx.enter_context(tc.tile_pool(name="psum", bufs=1, space="PSUM"))

        mmT_ps = psum.tile([16, nbags], f32, name="mmT_ps")[:]
        h1pa = psum.tile([128, nbags], f32, name="h1pa")[:]
        h1pb = psum.tile([72, nbags], f32, name="h1pb")[:]
        h2p = psum.tile([100, nbags], f32, name="h2p")[:]
        lp = psum.tile([1, nbags], f32, name="lp")[:]
        nc.tensor.transpose(mmT_ps, mm[:], idn_sb[:])
        mmT = tpool.tile([16, nbags], f32)
        nc.vector.tensor_copy(mmT[:], mmT_ps)

        nc.tensor.matmul(h1pa, lhsT=w1t_sb[:, 0:128], rhs=mmT[:], start=True, stop=True)
        nc.tensor.matmul(h1pb, lhsT=w1t_sb[:, 128:200], rhs=mmT[:], start=True, stop=True)
        h1a = tpool.tile([128, nbags], f32)
        h1b = tpool.tile([72, nbags], f32)
        nc.scalar.activation(h1a[:], h1pa, Act.Sigmoid, bias=b1a_sb[:], scale=1.0)
        nc.scalar.activation(h1b[:], h1pb, Act.Sigmoid, bias=b1b_sb[:], scale=1.0)

        nc.tensor.matmul(h2p, lhsT=w2ta_sb[:], rhs=h1a[:], start=True, stop=False)
        nc.tensor.matmul(h2p, lhsT=w2tb_sb[:], rhs=h1b[:], start=False, stop=True)
        h2 = tpool.tile([100, nbags], f32)
        nc.scalar.activation(h2[:], h2p, Act.Sigmoid, bias=b2c_sb[:], scale=1.0)

        nc.tensor.matmul(lp, lhsT=w3t_sb[:], rhs=h2[:], start=True, stop=True)
        outt = tpool.tile([1, 2 * nbags], f32)
        nc.vector.tensor_scalar_add(outt[:, 0:nbags], lp, b3c_sb[:])
        nc.scalar.activation(outt[:, nbags : 2 * nbags], outt[:, 0:nbags], Act.Sigmoid)

        nc.sync.dma_start(outlp[:], outt[:])

    nc.finalize()
    return nc


def _make_in_maps(inputs, nbags, ntiles, fsz, ncores):
    import ml_dtypes

    bf16 = ml_dtypes.bfloat16
    feats = np.asarray(inputs["feats"], dtype=np.float32)
    w_conv = np.asarray(inputs["w_conv"], dtype=np.float32)
    W1 = np.asarray(inputs["W1"], dtype=np.float32)
    b1 = np.asarray(inputs["b1"], dtype=np.float32)
    W2 = np.asarray(inputs["W2"], dtype=np.float32)
    b2 = np.asarray(inputs["b2"], dtype=np.float32)
    W3 = np.asarray(inputs["W3"], dtype=np.float32)
    b3 = np.asarray(inputs["b3"], dtype=np.float32)

    r_db, r_pb, srs = _split(ntiles)
    nfc = fsz // 128
    nchunk_d = r_db // 256
    supers = r_pb // srs

    # kernel x layout: x[k] = k-th largest score (k<8), x[8+j] = j-th largest
    # of negated scores = -(j-th smallest) (j<8).
    # reference minmax: [b0..b4 asc, t0..t4 asc] -> W1p columns:
    W1p = np.zeros((200, 16), dtype=np.float32)
    for k in range(5):
        W1p[:, k] = W1[:, 9 - k]          # t_(k) -> minmax[9-k]
    for j in range(5):
        W1p[:, 8 + j] = -W1[:, j]         # -b_j -> minmax[j]

    w16 = w_conv.astype(bf16)
    wrep = np.zeros((128, nfc * 128), dtype=bf16)
    for c in range(nfc):
        wrep[:, c * 128 : (c + 1) * 128] = w16[c * 128 : (c + 1) * 128][:, None]

    base = {
        "wb": np.ascontiguousarray(np.broadcast_to(w16, (128, fsz))),
        "wrep": wrep,
        "w1t": np.ascontiguousarray(W1p.T),
        "w2ta": np.ascontiguousarray(W2.T[:128]),
        "w2tb": np.ascontiguousarray(W2.T[128:]),
        "w3t": np.ascontiguousarray(W3.T),
        "b1a": np.ascontiguousarray(b1[:128].reshape(128, 1)),
        "b1b": np.ascontiguousarray(b1[128:].reshape(72, 1)),
        "b2c": np.ascontiguousarray(b2.reshape(100, 1)),
        "b3c": np.ascontiguousarray(b3.reshape(1, 1)),
        "idn": np.eye(nbags, dtype=np.float32),
    }
    feats16 = feats.astype(bf16)
    in_maps = []
    for cidx in range(ncores):
        shard = feats16[cidx * nbags : (cidx + 1) * nbags]  # [nbags, ntiles, fsz]
        fd = shard[:, :r_db, :].reshape(nbags * nchunk_d, 128, 4096)
        # PE part: [b, super, fchunk, 128 f, srs rows]
        fp = (shard[:, r_db:, :]
              .reshape(nbags, supers, srs, nfc, 128)
              .transpose(0, 1, 3, 4, 2)
              .reshape(nbags * supers * nfc, 128, srs))
        in_maps.append({
            **base,
            "feats_d": np.ascontiguousarray(fd),
            "feats_p": np.ascontiguousarray(fp),
        })
    return in_maps


def _run(inputs, trace=False, **spmd_kwargs):
    from concourse.bass_utils import run_bass_kernel_spmd

    nc = _build_nc(BAGS_PER_CORE, NTILES, FSZ)
    in_maps = _make_in_maps(inputs, BAGS_PER_CORE, NTILES, FSZ, NCORES)
    res = run_bass_kernel_spmd(
        nc, in_maps, list(range(NCORES)), trace=trace, **spmd_kwargs
    )
    logits = np.concatenate(
        [res.results[c]["outlp"].reshape(2, BAGS_PER_CORE)[0].reshape(-1, 1)
         for c in range(NCORES)],
        axis=0,
    )
    probs = np.concatenate(
        [res.results[c]["outlp"].reshape(2, BAGS_PER_CORE)[1].reshape(-1, 1)
         for c in range(NCORES)],
        axis=0,
    )
    return (logits, probs), res


def kernel(**inputs):
    out, _ = _run(inputs, trace=False)
    return out


# revision 20
# speedup vs baseline: 1.0286x; 1.0008x over previous
# Trainium2 Bass kernel for nn_Model_26190710571339 (topk_masking).
#
# Model: scores = einsum('bnf,f->bn', feats, w_conv); per-bag sort -> bottom-5
# and top-5 score values -> tiny MLP (10->200->100->1, sigmoid) -> logits, probs.
#
# Sharding: data-parallel over the bag axis; 2 bags per NeuronCore x 8 cores.
# Weights replicated.
#
# Hybrid DVE + PE scoring. feats are cast to bf16 on the host (halves the
# HBM stream to 134 MB/core; measured rel err 7e-4 vs the 2e-2 gate). The DVE
# fused multiply+reduce runs at 1x (2284 ns per 2048-wide tile), so DVE alone
# binds at ~585 us. To break that, each bag's 16384 rows are split on the
# host: 6144 rows keep the normal layout for DVE scalar_tensor_tensor; 10240
# rows are stored HOST-TRANSPOSED [16 fchunk, 128 f, rows] for the PE:
#   - stationary lhsT = w-chunk replicated into [128 f, 128] (every column
#     identical), moving rhs = [128 f, 512 rows] slab slices, PSUM
#     accumulating over the 16 f-chunks in TWO alternating banks (even/odd
#     chunks) to break the per-matmul PSUM turnaround stall
#   - every PSUM partition holds the same 512 partial scores; one ACT copy
#     of partition 0 per bank stages them, then two partition-unfold SWDGE
#     DMAs (second with CCE accum_op=add) scatter the summed scores
#     bijectively into the same [128, cols] scores tile the DVE writes ->
#     the topk/MLP tail is shared
# DVE ~220 us and PE ~330 us (incl ~450 ns/matmul waits) run in parallel;
# measured 394 us total (down from 598 us DVE-only).
#   - per-bag top-8/bottom-8 via single-instruction nc.vector.max (top-5 of a
#     bag is always inside the union of per-partition top-8), bottom side via
#     max over negated scores; bag 0 overlaps the bag-1 stream
#   - sort-order permutation + bottom-side sign flip folded into W1 on host
#   - MLP in transposed form; logits+probs packed into one [1, 4] output DMA
# (tensor_tensor_reduce crashes the device in this lowering; gpsimd cannot
# run stt [Pool engine opcode check]; bf16 stt runs at DVE 1x mode.)

import numpy as np

B = 16
NTILES = 16384
FSZ = 2048
R = 5
NCORES = 8
BAGS_PER_CORE = B // NCORES  # 2


def _split(ntiles):
    """Per-bag row split between the DVE and PE pipelines."""
    if ntiles >= 16384:
        return 6144, 10240, 2560
    r_pb = ntiles // 2
    return ntiles - r_pb, r_pb, 512


def _build_nc(nbags, ntiles, fsz, ncores=NCORES):
    import concourse.mybir as mybir
    import concourse.tile as tile
    from concourse import bacc
    from contextlib import ExitStack

    f32 = mybir.dt.float32
    bf16 = mybir.dt.bfloat16
    Alu = mybir.AluOpType
    Act = mybir.ActivationFunctionType

    r_db, r_pb, srs = _split(ntiles)
    nfc = fsz // 128              # f-chunks (16)
    nchunk_d = r_db // 256        # DVE 1MB chunks per bag
    supers = r_pb // srs          # PE supers per bag
    gps = srs // 512              # 512-row PSUM groups per super
    X = r_pb // 128               # PE score cols per partition
    assert srs % X == 0
    qps = srs // X                # score partitions covered per super
    cols_per_bag = ntiles // 128
    cols_d = r_db // 128          # DVE score cols per bag
    nblk = nbags * cols_per_bag

    nc = bacc.Bacc("TRN2", target_bir_lowering=False, debug=False, num_devices=ncores)
    feats_d = nc.declare_dram_parameter(
        "feats_d", [nbags * nchunk_d, 128, 4096], bf16, isOutput=False)
    feats_p = nc.declare_dram_parameter(
        "feats_p", [nbags * supers * nfc, 128, srs], bf16, isOutput=False)
    wb = nc.declare_dram_parameter("wb", [128, fsz], bf16, isOutput=False)
    wrep = nc.declare_dram_parameter("wrep", [128, nfc * 128], bf16, isOutput=False)
    w1t = nc.declare_dram_parameter("w1t", [16, 200], f32, isOutput=False)
    w2ta = nc.declare_dram_parameter("w2ta", [128, 100], f32, isOutput=False)
    w2tb = nc.declare_dram_parameter("w2tb", [72, 100], f32, isOutput=False)
    w3t = nc.declare_dram_parameter("w3t", [100, 1], f32, isOutput=False)
    b1a = nc.declare_dram_parameter("b1a", [128, 1], f32, isOutput=False)
    b1b = nc.declare_dram_parameter("b1b", [72, 1], f32, isOutput=False)
    b2c = nc.declare_dram_parameter("b2c", [100, 1], f32, isOutput=False)
    b3c = nc.declare_dram_parameter("b3c", [1, 1], f32, isOutput=False)
    idn = nc.declare_dram_parameter("idn", [nbags, nbags], f32, isOutput=False)
    outlp = nc.declare_dram_parameter("outlp", [1, 2 * nbags], f32, isOutput=True)

    dma_rr = [0]

    def next_ring():
        dma_rr[0] += 1
        return nc.sync if dma_rr[0] % 2 == 0 else nc.scalar

    with ExitStack() as ctx:
        tc = ctx.enter_context(tile.TileContext(nc))
        consts = ctx.enter_context(tc.tile_pool(name="consts", bufs=1))

        wb_sb = consts.tile([128, fsz], bf16)
        nc.sync.dma_start(wb_sb[:], wb[:])
        wrep_sb = consts.tile([128, nfc * 128], bf16)
        nc.scalar.dma_start(wrep_sb[:], wrep[:])
        scores = consts.tile([128, nblk], f32)

        # per-bag selection state
        tpool = ctx.enter_context(tc.tile_pool(name="tpool", bufs=1))
        neg = tpool.tile([128, cols_per_bag], f32)
        gsrc = [tpool.tile([128, 16], f32, name=f"gsrc{b}") for b in range(nbags)]
        cand_top = tpool.tile([nbags, 128 * 8], f32)
        cand_bot = tpool.tile([nbags, 128 * 8], f32)
        mm = tpool.tile([nbags, 16], f32)

        def emit_bag_topk(b):
            sc_b = scores[:, b * cols_per_bag : (b + 1) * cols_per_bag]
            # top-8 per partition; bottom-8 via max over negated scores
            nc.vector.max(gsrc[b][:, 0:8], sc_b)
            nc.vector.tensor_scalar_mul(neg[:], sc_b, -1.0)
            nc.vector.max(gsrc[b][:, 8:16], neg[:])
            # gather the 128x8 candidates of each side into one partition row
            nc.scalar.dma_start(cand_top[b : b + 1, :], gsrc[b][:, 0:8])
            nc.sync.dma_start(cand_bot[b : b + 1, :], gsrc[b][:, 8:16])

        fpool = ctx.enter_context(tc.tile_pool(name="fpool", bufs=4))
        opool = ctx.enter_context(tc.tile_pool(name="opool", bufs=4))
        spool = ctx.enter_context(tc.tile_pool(name="spool", bufs=18))
        prpool = ctx.enter_context(tc.tile_pool(name="prpool", bufs=2))
        pe_psum = ctx.enter_context(
            tc.tile_pool(name="pe_psum", bufs=3, space="PSUM"))

        def emit_dve_chunk(b, k):
            ft = fpool.tile([128, 4096], bf16, name="ft")
            next_ring().dma_start(ft[:], feats_d[b * nchunk_d + k])
            for h in range(2):
                col = b * cols_per_bag + 2 * k + h
                ot = opool.tile([128, fsz], bf16, name="ot")
                nc.vector.scalar_tensor_tensor(
                    out=ot[:],
                    in0=ft[:, h * fsz : (h + 1) * fsz],
                    scalar=1.0,
                    in1=wb_sb[:],
                    op0=Alu.mult,
                    op1=Alu.mult,
                    accum_out=scores[:, col : col + 1],
                )

        def emit_pe_super(b, s):
            slabs = []
            for c in range(nfc):
                sl = spool.tile([128, srs], bf16, name="sl")
                next_ring().dma_start(sl[:], feats_p[(b * supers + s) * nfc + c])
                slabs.append(sl)
            # group-outer: only group 0's c-walk chases the slab DMAs; later
            # groups re-read resident slabs with no waits. Within a group the
            # 16 accumulating matmuls alternate TWO psum banks (even/odd
            # f-chunks) to break the per-matmul PSUM turnaround stall; the
            # two partial sums are added by the scatter DMA (CCE accum).
            pra = prpool.tile([1, gps * 512], f32, name="pra")
            prb = prpool.tile([1, gps * 512], f32, name="prb")
            for g in range(gps):
                psa = pe_psum.tile([128, 512], f32, name="ps")
                psb = pe_psum.tile([128, 512], f32, name="ps")
                for c in range(nfc):
                    ps = psa if c % 2 == 0 else psb
                    nc.tensor.matmul(
                        ps[:], lhsT=wrep_sb[:, c * 128 : (c + 1) * 128],
                        rhs=slabs[c][:, g * 512 : (g + 1) * 512],
                        start=(c < 2), stop=(c >= nfc - 2),
                    )
                # every PSUM partition holds the group's 512 partial scores;
                # stage partition 0 of each bank into the super's SBUF rows
                nc.scalar.activation(
                    pra[:, g * 512 : (g + 1) * 512], psa[0:1, :], Act.Copy)
                nc.scalar.activation(
                    prb[:, g * 512 : (g + 1) * 512], psb[0:1, :], Act.Copy)
            # two partition-unfold DMAs scatter the super's gps*512 scores
            # over partitions [qps*s, qps*(s+1)) x X cols of this bag's PE
            # score region; the second adds the odd-chunk partial in the
            # DMA datapath.
            dst = scores[qps * s : qps * (s + 1),
                         b * cols_per_bag + cols_d :
                         b * cols_per_bag + cols_d + X]
            nc.gpsimd.dma_start(dst, pra[:])
            nc.gpsimd.dma_start(dst, prb[:], accum_op=Alu.add)

        # ---- main loop: interleave DVE chunks and PE supers per bag
        cps = nchunk_d // supers  # DVE chunks emitted per PE super
        for b in range(nbags):
            for s in range(supers):
                for k in range(s * cps, (s + 1) * cps):
                    emit_dve_chunk(b, k)
                emit_pe_super(b, s)
            emit_bag_topk(b)

        w1t_sb = consts.tile([16, 200], f32)
        nc.sync.dma_start(w1t_sb[:], w1t[:])
        w2ta_sb = consts.tile([128, 100], f32)
        nc.sync.dma_start(w2ta_sb[:], w2ta[:])
        w2tb_sb = consts.tile([72, 100], f32)
        nc.sync.dma_start(w2tb_sb[:], w2tb[:])
        w3t_sb = consts.tile([100, 1], f32)
        nc.sync.dma_start(w3t_sb[:], w3t[:])
        b1a_sb = consts.tile([128, 1], f32)
        nc.sync.dma_start(b1a_sb[:], b1a[:])
        b1b_sb = consts.tile([72, 1], f32)
        nc.sync.dma_start(b1b_sb[:], b1b[:])
        b2c_sb = consts.tile([100, 1], f32)
        nc.sync.dma_start(b2c_sb[:], b2c[:])
        b3c_sb = consts.tile([1, 1], f32)
        nc.sync.dma_start(b3c_sb[:], b3c[:])
        idn_sb = consts.tile([nbags, nbags], f32)
        nc.sync.dma_start(idn_sb[:], idn[:])

        # ---- global top/bottom-8 per bag (rows = bags); top-5 subset exact
        nc.vector.max(mm[:, 0:8], cand_top[:])
        nc.vector.max(mm[:, 8:16], cand_bot[:])

        # ---- MLP (transposed): hT = sigmoid(W @ xT + b), biases per-partition.
        # mm columns = [top8 desc | (-bottom)8 desc]; W1 permuted/sign-flipped
        # on the host to match, so this equals the reference MLP.
        psum = ctx.enter_context(tc.tile_pool(name="psum", bufs=1, space="PSUM"))

        mmT_ps = psum.tile([16, nbags], f32, name="mmT_ps")[:]
        h1pa = psum.tile([128, nbags], f32, name="h1pa")[:]
        h1pb = psum.tile([72, nbags], f32, name="h1pb")[:]
        h2p = psum.tile([100, nbags], f32, name="h2p")[:]
        lp = psum.tile([1, nbags], f32, name="lp")[:]
        nc.tensor.transpose(mmT_ps, mm[:], idn_sb[:])
        mmT = tpool.tile([16, nbags], f32)
        nc.vector.tensor_copy(mmT[:], mmT_ps)

        nc.tensor.matmul(h1pa, lhsT=w1t_sb[:, 0:128], rhs=mmT[:], start=True, stop=True)
        nc.tensor.matmul(h1pb, lhsT=w1t_sb[:, 128:200], rhs=mmT[:], start=True, stop=True)
        h1a = tpool.tile([128, nbags], f32)
        h1b = tpool.tile([72, nbags], f32)
        nc.scalar.activation(h1a[:], h1pa, Act.Sigmoid, bias=b1a_sb[:], scale=1.0)
        nc.scalar.activation(h1b[:], h1pb, Act.Sigmoid, bias=b1b_sb[:], scale=1.0)

        nc.tensor.matmul(h2p, lhsT=w2ta_sb[:], rhs=h1a[:], start=True, stop=False)
        nc.tensor.matmul(h2p, lhsT=w2tb_sb[:], rhs=h1b[:], start=False, stop=True)
        h2 = tpool.tile([100, nbags], f32)
        nc.scalar.activation(h2[:], h2p, Act.Sigmoid, bias=b2c_sb[:], scale=1.0)

        nc.tensor.matmul(lp, lhsT=w3t_sb[:], rhs=h2[:], start=True, stop=True)
        outt = tpool.tile([1, 2 * nbags], f32)
        nc.vector.tensor_scalar_add(outt[:, 0:nbags], lp, b3c_sb[:])
        nc.scalar.activation(outt[:, nbags : 2 * nbags], outt[:, 0:nbags], Act.Sigmoid)

        nc.sync.dma_start(outlp[:], outt[:])

    nc.finalize()
    return nc


def _make_in_maps(inputs, nbags, ntiles, fsz, ncores):
    import ml_dtypes

    bf16 = ml_dtypes.bfloat16
    feats = np.asarray(inputs["feats"], dtype=np.float32)
    w_conv = np.asarray(inputs["w_conv"], dtype=np.float32)
    W1 = np.asarray(inputs["W1"], dtype=np.float32)
    b1 = np.asarray(inputs["b1"], dtype=np.float32)
    W2 = np.asarray(inputs["W2"], dtype=np.float32)
    b2 = np.asarray(inputs["b2"], dtype=np.float32)
    W3 = np.asarray(inputs["W3"], dtype=np.float32)
    b3 = np.asarray(inputs["b3"], dtype=np.float32)

    r_db, r_pb, srs = _split(ntiles)
    nfc = fsz // 128
    nchunk_d = r_db // 256
    supers = r_pb // srs

    # kernel x layout: x[k] = k-th largest score (k<8), x[8+j] = j-th largest
    # of negated scores = -(j-th smallest) (j<8).
    # reference minmax: [b0..b4 asc, t0..t4 asc] -> W1p columns:
    W1p = np.zeros((200, 16), dtype=np.float32)
    for k in range(5):
        W1p[:, k] = W1[:, 9 - k]          # t_(k) -> minmax[9-k]
    for j in range(5):
        W1p[:, 8 + j] = -W1[:, j]         # -b_j -> minmax[j]

    w16 = w_conv.astype(bf16)
    wrep = np.zeros((128, nfc * 128), dtype=bf16)
    for c in range(nfc):
        wrep[:, c * 128 : (c + 1) * 128] = w16[c * 128 : (c + 1) * 128][:, None]

    base = {
        "wb": np.ascontiguousarray(np.broadcast_to(w16, (128, fsz))),
        "wrep": wrep,
        "w1t": np.ascontiguousarray(W1p.T),
        "w2ta": np.ascontiguousarray(W2.T[:128]),
        "w2tb": np.ascontiguousarray(W2.T[128:]),
        "w3t": np.ascontiguousarray(W3.T),
        "b1a": np.ascontiguousarray(b1[:128].reshape(128, 1)),
        "b1b": np.ascontiguousarray(b1[128:].reshape(72, 1)),
        "b2c": np.ascontiguousarray(b2.reshape(100, 1)),
        "b3c": np.ascontiguousarray(b3.reshape(1, 1)),
        "idn": np.eye(nbags, dtype=np.float32),
    }
    feats16 = feats.astype(bf16)
    in_maps = []
    for cidx in range(ncores):
        shard = feats16[cidx * nbags : (cidx + 1) * nbags]  # [nbags, ntiles, fsz]
        fd = shard[:, :r_db, :].reshape(nbags * nchunk_d, 128, 4096)
        # PE part: [b, super, fchunk, 128 f, srs rows]
        fp = (shard[:, r_db:, :]
              .reshape(nbags, supers, srs, nfc, 128)
              .transpose(0, 1, 3, 4, 2)
              .reshape(nbags * supers * nfc, 128, srs))
        in_maps.append({
            **base,
            "feats_d": np.ascontiguousarray(fd),
            "feats_p": np.ascontiguousarray(fp),
        })
    return in_maps


def _run(inputs, trace=False, **spmd_kwargs):
    from concourse.bass_utils import run_bass_kernel_spmd

    nc = _build_nc(BAGS_PER_CORE, NTILES, FSZ)
    in_maps = _make_in_maps(inputs, BAGS_PER_CORE, NTILES, FSZ, NCORES)
    res = run_bass_kernel_spmd(
        nc, in_maps, list(range(NCORES)), trace=trace, **spmd_kwargs
    )
    logits = np.concatenate(
        [res.results[c]["outlp"].reshape(2, BAGS_PER_CORE)[0].reshape(-1, 1)
         for c in range(NCORES)],
        axis=0,
    )
    probs = np.concatenate(
        [res.results[c]["outlp"].reshape(2, BAGS_PER_CORE)[1].reshape(-1, 1)
         for c in range(NCORES)],
        axis=0,
    )
    return (logits, probs), res


def kernel(**inputs):
    out, _ = _run(inputs, trace=False)
    return out
